# revision 9
# baseline (speedup 1.0000x reference)
"""Trainium2 Bass kernel for a 3-step conditioned GraphNets MetaLayer pair.

Problem structure (hardcoded, matches the generator):
  B=2048 graphs, 16 nodes/graph (block-contiguous), complete digraph per
  graph (240 edges, src-major lexicographic order), H=HU=64, 3 steps.

Strategy: data-parallel over graphs, 256 graphs/core on 8 cores.
On-device layout is "graph-paired transposed": 128 SBUF partitions hold
features of two consecutive graphs (rows 0:64 = even graph, 64:128 = odd
graph); columns enumerate edges/nodes/graphs.  Edges use a per-graph
256-slot grid (slot = 16*dst_local + src_local, diagonal unused) so that
gathers/scatters become structured access patterns:
  - x[dst]-x[src] terms become a matmul against a constant +/-1 selector
  - per-node incoming-edge means become 16 strided accumulating matmuls
  - per-graph means become pool_avg over contiguous windows
The inter-step edge state stored to HBM is R2 = relu(hidden2) with the
last MLP layer (W3, b3) folded into every consumer's weights/biases, which
removes a full matmul pass and a full PSUM-drain pass per meta-step.
All matmuls run as fp32r (TF32-like) by default.
"""

import sys
import os
from contextlib import ExitStack

sys.path.insert(0, "/opt/trn_rl_repo")

import numpy as np

import concourse.bass as bass
import concourse.bacc as bacc
import concourse.tile as tile
from concourse import mybir
from concourse.bass_utils import run_bass_kernel_spmd

# ----------------------------------------------------------------- constants
B = 2048
NPER = 16
H = 64
EPG = NPER * (NPER - 1)      # 240
GRID = NPER * NPER           # 256
NSTEPS = 3
F_OUT = 2
NCORES = 8
GPC = B // NCORES            # graphs per core = 256
P_FULL = GPC // 2            # pairs per core = 128

f32 = mybir.dt.float32
f32r = mybir.dt.float32r
bf16 = mybir.dt.bfloat16

AOT = mybir.AluOpType
AFT = mybir.ActivationFunctionType

# compute/storage dtype for activations & weights on device
DT = f32r
DT_NP = np.float32          # numpy dtype backing DT in DRAM

CHUNK = 512                  # grid columns per matmul (2 pairs)
PPE = 16                     # pairs per edge tile (AGG group)


# ------------------------------------------------------- host index helpers
def _grid_slots():
    """packed edge order -> grid slot (16*dst + src)."""
    slots = []
    for i in range(NPER):
        for j in range(NPER):
            if i != j:
                slots.append(16 * j + i)
    return np.asarray(slots, np.int64)


_SLOTS = _grid_slots()


def pack_edges(e, n_cores=NCORES):
    """[nb*240, H] -> per-core [128, (nb/n_cores/2)*256], paired grid."""
    nb = e.shape[0] // EPG
    gpc = nb // n_cores
    eg = np.zeros((nb, GRID, H), np.float32)
    eg[:, _SLOTS, :] = e.reshape(nb, EPG, H)
    out = []
    for c in range(n_cores):
        g = eg[c * gpc:(c + 1) * gpc]                       # [gpc, 256, H]
        g = g.reshape(gpc // 2, 2, GRID, H).transpose(1, 3, 0, 2)
        out.append(np.ascontiguousarray(g.reshape(2 * H, (gpc // 2) * GRID)))
    return out


def pack_nodes(x, n_cores=NCORES):
    """[nb*16, H] -> per-core [128, (nb/n_cores/2)*16]."""
    nb = x.shape[0] // NPER
    gpc = nb // n_cores
    out = []
    for c in range(n_cores):
        g = x.reshape(nb, NPER, H)[c * gpc:(c + 1) * gpc]
        g = g.reshape(gpc // 2, 2, NPER, H).transpose(1, 3, 0, 2)
        out.append(np.ascontiguousarray(g.reshape(2 * H, (gpc // 2) * NPER)))
    return out


def pack_globals(u, n_cores=NCORES):
    """[nb, H] -> per-core [128, nb/n_cores/2]."""
    gpc = u.shape[0] // n_cores
    out = []
    for c in range(n_cores):
        g = u[c * gpc:(c + 1) * gpc]
        g = g.reshape(gpc // 2, 2, H).transpose(1, 2, 0)
        out.append(np.ascontiguousarray(g.reshape(2 * H, gpc // 2)))
    return out


def bd2(w):
    """[64, m] -> [128, 2m] block-diagonal (paired lhsT)."""
    k, m = w.shape
    o = np.zeros((2 * k, 2 * m), np.float32)
    o[:k, :m] = w
    o[k:, m:] = w
    return o


def pvec(b):
    """[64] -> [128, 1] paired bias column."""
    return np.concatenate([b, b]).reshape(2 * H, 1).astype(np.float32)


def sel_y():
    """[128, 2048] 8-pair block-diag +/-1 Y selector (rows = 8x16 nodes)."""
    s = np.zeros((NPER, GRID), np.float32)
    for q in range(GRID):
        j, i = q // 16, q % 16
        s[j, q] += 1.0
        s[i, q] -= 1.0
    out = np.zeros((128, 8 * GRID), np.float32)
    for m in range(8):
        out[16 * m:16 * m + 16, GRID * m:GRID * m + GRID] = s
    return out


def sel_n():
    """[128, 2048] pair -> its 16 node-columns broadcast selector."""
    s = np.zeros((128, 128 * NPER), np.float32)
    for r in range(128):
        s[r, 16 * r:16 * r + 16] = 1.0
    return s


def make_consts(edge_q, node_q, global_q, edge_w, node_w, global_w,
                resize_w, resize_b, readout):
    """Host-precomputed constant tensors (shared by all cores)."""
    def unpack(mlp):
        (w1, b1), (w2, b2), (w3, b3) = mlp
        return (np.asarray(w1, np.float32), np.asarray(b1, np.float32),
                np.asarray(w2, np.float32), np.asarray(b2, np.float32),
                np.asarray(w3, np.float32), np.asarray(b3, np.float32))

    eW1, eb1, eW2, eb2, eW3, eb3 = unpack(edge_q)
    nW1, nb1, nW2, nb2, nW3, nb3 = unpack(node_q)
    gW1, gb1, gW2, gb2, gW3, gb3 = unpack(global_q)
    e2W1, e2b1, e2W2, e2b2, e2W3, e2b3 = unpack(edge_w)
    n2W1, n2b1, n2W2, n2b2, n2W3, n2b3 = unpack(node_w)
    g2W1, g2b1, g2W2, g2b2, g2W3, g2b3 = unpack(global_w)
    roW1, rob1, roW2, rob2, roW3, rob3 = unpack(readout)
    rsW = np.asarray(resize_w, np.float32)
    rsb = np.asarray(resize_b, np.float32)

    C = {}
    # ---- meta-1 (query graph), edge_q input = [e, dx, u]
    W1e, W1dx, W1u = eW1[0:64], eW1[64:128], eW1[128:192]
    C["m1_A_s1"] = bd2(W1e)
    C["m1_A_s23"] = bd2(eW3 @ W1e)
    C["m1_DX"] = bd2(W1dx)
    C["m1_CU"] = bd2(W1u)
    C["m1_W2"] = bd2(eW2)
    C["m1_cb_s1"] = pvec(eb1)
    C["m1_cb_s23"] = pvec(eb1 + W1e.T @ eb3)
    C["m1_b2"] = pvec(eb2)
    # node_q input = [x, agg, u]
    Wnx, Wnagg, Wnu = nW1[0:64], nW1[64:128], nW1[128:192]
    C["m1_NX"] = bd2(Wnx)
    C["m1_NAGG"] = bd2(eW3 @ Wnagg)
    C["m1_NU"] = bd2(Wnu)
    C["m1_NW2"] = bd2(nW2)
    C["m1_NW3"] = bd2(nW3)
    C["m1_nb1_s1"] = pvec(nb1 + Wnagg.T @ eb3)
    C["m1_nb1_s23"] = pvec(nb1 + Wnagg.T @ eb3 + Wnx.T @ nb3)
    C["m1_nb2"] = pvec(nb2)
    # global_q input = [em, xm, u]
    Wgem, Wgxm, Wgu = gW1[0:64], gW1[64:128], gW1[128:192]
    C["m1_GEM"] = bd2(eW3 @ Wgem / 16.0)
    C["m1_GXM"] = bd2(Wgxm / 16.0)
    C["m1_GU"] = bd2(Wgu)
    C["m1_GW2"] = bd2(gW2)
    C["m1_GW3"] = bd2(gW3)
    C["m1_gb1"] = pvec(gb1 + Wgem.T @ eb3 + Wgxm.T @ nb3)
    C["m1_gb2"] = pvec(gb2)
    C["m1_gb3"] = pvec(gb3)

    # ---- meta-2 (world graph), edge_w input = [e2c, dx2c, u2c]
    #   e2c=[e2; u1], dx2c=[dx2; 0], u2c=[u2; u1]
    W1e2 = e2W1[0:64]
    Wc_u1 = e2W1[64:128] + e2W1[320:384]
    W1dx2 = e2W1[128:192]
    Wc_u2 = e2W1[256:320]
    C["m2_A_s1"] = bd2(W1e2)
    C["m2_A_s23"] = bd2(e2W3 @ W1e2)
    C["m2_DX"] = bd2(W1dx2)
    C["m2_CU1"] = bd2(Wc_u1)
    C["m2_CU2"] = bd2(Wc_u2)
    C["m2_W2"] = bd2(e2W2)
    C["m2_cb_s1"] = pvec(e2b1)
    C["m2_cb_s23"] = pvec(e2b1 + W1e2.T @ e2b3)
    C["m2_b2"] = pvec(e2b2)
    # node_w input = [x2c, agg2, u2c] = [[x2; u1], agg2, [u2; u1]]
    Wnx2 = n2W1[0:64]
    Wn2_u1 = n2W1[64:128] + n2W1[256:320]
    Wnagg2 = n2W1[128:192]
    Wn2_u2 = n2W1[192:256]
    C["m2_NX"] = bd2(Wnx2)
    C["m2_NAGG"] = bd2(e2W3 @ Wnagg2)
    C["m2_NU1"] = bd2(Wn2_u1)
    C["m2_NU2"] = bd2(Wn2_u2)
    C["m2_NW2"] = bd2(n2W2)
    C["m2_NW3"] = bd2(n2W3)
    C["m2_nb1_s1"] = pvec(n2b1 + Wnagg2.T @ e2b3)
    C["m2_nb1_s23"] = pvec(n2b1 + Wnagg2.T @ e2b3 + Wnx2.T @ n2b3)
    C["m2_nb2"] = pvec(n2b2)
    # global_w input = [em2, xm2, u2c]
    Wgem2, Wgxm2 = g2W1[0:64], g2W1[64:128]
    Wg2_u2, Wg2_u1 = g2W1[128:192], g2W1[192:256]
    C["m2_GEM"] = bd2(e2W3 @ Wgem2 / 16.0)
    C["m2_GXM"] = bd2(Wgxm2 / 16.0)
    C["m2_GU1"] = bd2(Wg2_u1)
    C["m2_GU2"] = bd2(Wg2_u2)
    C["m2_GW2"] = bd2(g2W2)
    C["m2_GW3"] = bd2(g2W3)
    C["m2_gb1"] = pvec(g2b1 + Wgem2.T @ e2b3 + Wgxm2.T @ n2b3)
    C["m2_gb2"] = pvec(g2b2)
    C["m2_gb3"] = pvec(g2b3)

    # ---- tail
    C["RSZ"] = bd2(rsW)
    C["rsb"] = pvec(rsb)
    C["RO1"] = bd2(roW1)
    C["RO2"] = bd2(roW2)
    C["RO3"] = bd2(roW3)                       # [128, 4]
    C["rob1"] = pvec(rob1)
    C["rob2"] = pvec(rob2)
    C["rob3"] = np.concatenate([rob3, rob3]).reshape(2 * F_OUT, 1).astype(np.float32)
    C["SELY"] = sel_y()
    C["SELN"] = sel_n()
    C["I15"] = (np.eye(128) / 15.0).astype(np.float32)
    C["I15N"] = (-np.eye(128) / 15.0).astype(np.float32)
    return C


# --------------------------------------------------------------- the kernel
def build_kernel(nc, tc, ctx, P):
    """Emit the full 3-step program for one core handling 2P graphs."""
    npairs_nt = min(8, P)            # pairs per node tile
    ppe = min(PPE, P)                # pairs per edge tile
    n_et = P // ppe
    n_nt = P // npairs_nt
    chunks_per_et = (ppe * GRID) // CHUNK
    pairs_per_chunk = CHUNK // GRID  # 2

    io = {}

    def ext_in(name, shape, dtype=f32):
        io[name] = nc.dram_tensor(name, list(shape), dtype, kind="ExternalInput").ap()
        return io[name]

    # inputs
    e1g = ext_in("e1g", (128, P * GRID))
    e2g = ext_in("e2g", (128, P * GRID))
    x1g = ext_in("x1g", (128, P * NPER))
    x2g = ext_in("x2g", (128, P * NPER))
    u1g = ext_in("u1g", (128, P))
    u2g = ext_in("u2g", (128, P))
    # const inputs (names must match make_consts keys)
    const_shapes = {
        "m1_A_s1": (128, 128), "m1_A_s23": (128, 128), "m1_DX": (128, 128),
        "m1_CU": (128, 128), "m1_W2": (128, 128),
        "m1_cb_s1": (128, 1), "m1_cb_s23": (128, 1), "m1_b2": (128, 1),
        "m1_NX": (128, 128), "m1_NAGG": (128, 128), "m1_NU": (128, 128),
        "m1_NW2": (128, 128), "m1_NW3": (128, 128),
        "m1_nb1_s1": (128, 1), "m1_nb1_s23": (128, 1), "m1_nb2": (128, 1),
        "m1_GEM": (128, 128), "m1_GXM": (128, 128), "m1_GU": (128, 128),
        "m1_GW2": (128, 128), "m1_GW3": (128, 128),
        "m1_gb1": (128, 1), "m1_gb2": (128, 1), "m1_gb3": (128, 1),
        "m2_A_s1": (128, 128), "m2_A_s23": (128, 128), "m2_DX": (128, 128),
        "m2_CU1": (128, 128), "m2_CU2": (128, 128), "m2_W2": (128, 128),
        "m2_cb_s1": (128, 1), "m2_cb_s23": (128, 1), "m2_b2": (128, 1),
        "m2_NX": (128, 128), "m2_NAGG": (128, 128), "m2_NU1": (128, 128),
        "m2_NU2": (128, 128), "m2_NW2": (128, 128), "m2_NW3": (128, 128),
        "m2_nb1_s1": (128, 1), "m2_nb1_s23": (128, 1), "m2_nb2": (128, 1),
        "m2_GEM": (128, 128), "m2_GXM": (128, 128), "m2_GU1": (128, 128),
        "m2_GU2": (128, 128), "m2_GW2": (128, 128), "m2_GW3": (128, 128),
        "m2_gb1": (128, 1), "m2_gb2": (128, 1), "m2_gb3": (128, 1),
        "RSZ": (128, 128), "rsb": (128, 1),
        "RO1": (128, 128), "RO2": (128, 128), "RO3": (128, 4),
        "rob1": (128, 1), "rob2": (128, 1), "rob3": (2 * F_OUT, 1),
        "SELY": (128, 8 * GRID), "SELN": (128, 128 * NPER),
        "I15": (128, 128), "I15N": (128, 128),
    }
    for nm, shp in const_shapes.items():
        ext_in(nm, shp)

    out_d = nc.dram_tensor("out", [2 * F_OUT, NSTEPS * P], f32,
                           kind="ExternalOutput").ap()
    io["out"] = out_d

    # scratch (internal DRAM), ping-pong per edge family
    scr = {(fam, pp): nc.dram_tensor(f"scr{fam}{pp}", [128, P * GRID], DT).ap()
           for fam in (1, 2) for pp in ("a", "b")}

    # ------------------------------------------------------------ SBUF pools
    res = ctx.enter_context(tc.tile_pool(name="res", bufs=1))
    wpool = ctx.enter_context(tc.tile_pool(name="w", bufs=1))
    etp = ctx.enter_context(tc.tile_pool(name="et", bufs=2))
    r1p = ctx.enter_context(tc.tile_pool(name="r1", bufs=3))
    pL1 = ctx.enter_context(tc.tile_pool(name="pL1", bufs=2, space="PSUM"))
    pL2 = ctx.enter_context(tc.tile_pool(name="pL2", bufs=2, space="PSUM"))
    pAgg = ctx.enter_context(tc.tile_pool(name="pAgg", bufs=2, space="PSUM"))
    pSm = ctx.enter_context(tc.tile_pool(name="pSm", bufs=2, space="PSUM"))

    # load constants to SBUF
    W = {}
    for nm, shp in const_shapes.items():
        dt_ = f32 if shp[1] == 1 else DT
        t = wpool.tile(list(shp), dt_, tag=nm, name="w_" + nm)
        src = io[nm]
        nc.sync.dma_start(t[:], src[:].bitcast(dt_) if dt_ != f32 else src[:])
        W[nm] = t

    # resident tiles
    def rtile(name, cols, dt_=DT):
        return res.tile([128, cols], dt_, tag=name, name=name)

    X = {1: [rtile("x1a", P * NPER), rtile("x1b", P * NPER)],
         2: [rtile("x2a", P * NPER), rtile("x2b", P * NPER)]}
    U1 = [rtile("u1a", P), rtile("u1b", P)]
    U2 = [rtile("u2a", P), rtile("u2b", P)]
    U1new = rtile("u1new", P)
    U1rel = rtile("u1rel", P)
    YS = rtile("ys", n_nt * 128)
    CTS = rtile("cts", P, f32)
    CnNS = rtile("cnns", 128)
    AGGS = rtile("aggs", P * NPER)
    EMRS = rtile("emrs", P)
    XMRS = rtile("xmrs", P)
    R2 = [rtile("r2a", ppe * GRID), rtile("r2b", ppe * GRID)]
    OUT = res.tile([2 * F_OUT, NSTEPS * P], f32, tag="outt", name="outt")

    # initial loads
    nc.sync.dma_start(X[1][0][:], x1g[:].bitcast(DT))
    nc.sync.dma_start(X[2][0][:], x2g[:].bitcast(DT))
    nc.sync.dma_start(U1[0][:], u1g[:].bitcast(DT))
    nc.sync.dma_start(U2[0][:], u2g[:].bitcast(DT))

    mm = nc.tensor.matmul
    drain_ct = [0]

    def drain(out_ap, in_ap, bias=None, relu=False):
        """PSUM -> SBUF with optional per-partition bias and relu.
        Alternates DVE / ACT for throughput."""
        use_act = (drain_ct[0] % 2 == 1) and relu
        drain_ct[0] += 1
        if use_act:
            nc.scalar.activation(out_ap, in_ap, AFT.Relu,
                                 bias=(bias[:, 0:1] if bias is not None else 0.0),
                                 scale=1.0)
        else:
            s1 = bias[:, 0:1] if bias is not None else 0.0
            if relu:
                nc.vector.tensor_scalar(out_ap, in_ap, s1, 0.0, AOT.add, AOT.max)
            else:
                nc.vector.tensor_scalar(out_ap, in_ap, s1, None, AOT.add)

    # --------------------------------------------------------------- 1 meta
    def meta(step, fam, m, Xcur, Xnew, Uins, Unew_t, egrid_in, scr_out):
        """One MetaLayer. m = 'm1'/'m2'; Uins = list of (Utile, CUw, NUw, GUw)."""
        s23 = step > 0
        # --- CTS: per-pair edge constant term (transposed) + bias
        cts_p = pSm.tile([128, P], f32, tag="sm")
        for k, (ut, cu, _, _) in enumerate(Uins):
            mm(cts_p[:], W[cu][:], ut[:], start=(k == 0),
               stop=(k == len(Uins) - 1))
        drain(CTS[:], cts_p[:], bias=W[f"{m}_cb_s23" if s23 else f"{m}_cb_s1"])

        # --- Cn (natural, per-pair node constant term)
        cn_p = pSm.tile([128, 128], f32, tag="sm")
        for k, (ut, _, nu, _) in enumerate(Uins):
            mm(cn_p[:P, :], ut[:], W[nu][:], start=(k == 0),
               stop=(k == len(Uins) - 1))
        drain(CnNS[:P, :], cn_p[:P, :])

        # --- Y tiles (natural paired: rows = nodes of 8 pairs, cols = feats)
        for nt in range(n_nt):
            ncol = npairs_nt * NPER
            y_p = pSm.tile([128, 128], f32, tag="sm")
            mm(y_p[:ncol, :], Xcur[:, nt * ncol:(nt + 1) * ncol],
               W[f"{m}_DX"][:], start=True, stop=True)
            drain(YS[:ncol, nt * 128:nt * 128 + 128], y_p[:ncol, :])

        # --- edge loop
        A = W[f"{m}_A_s23" if s23 else f"{m}_A_s1"]
        for t in range(n_et):
            et = etp.tile([128, ppe * GRID], DT, tag="et")
            nc.sync.dma_start(et[:], egrid_in[:, t * ppe * GRID:(t + 1) * ppe * GRID])
            r2t = R2[t % 2]
            for c in range(chunks_per_et):
                sl = slice(c * CHUNK, (c + 1) * CHUNK)
                pair0 = t * ppe + c * pairs_per_chunk
                l1 = pL1.tile([128, CHUNK], f32, tag="l1")
                mm(l1[:], A[:], et[:, sl], start=True, stop=False)
                nt = pair0 // npairs_nt
                krows = 16 * npairs_nt
                ysl = YS[:krows, nt * 128:nt * 128 + 128]
                selc = (pair0 % npairs_nt) * GRID
                mm(l1[:], ysl, W["SELY"][:krows, selc:selc + CHUNK],
                   start=False, stop=True)
                r1 = r1p.tile([128, CHUNK], DT, tag="r1")
                for pp in range(pairs_per_chunk):
                    psl = slice(pp * GRID, (pp + 1) * GRID)
                    drain(r1[:, psl], l1[:, psl],
                          bias=CTS[:, pair0 + pp:pair0 + pp + 1], relu=True)
                l2 = pL2.tile([128, CHUNK], f32, tag="l2")
                mm(l2[:], W[f"{m}_W2"][:], r1[:], start=True, stop=True)
                drain(r2t[:, sl], l2[:], bias=W[f"{m}_b2"], relu=True)
            # aggregation over this tile's ppe pairs (means w/ diag excluded)
            agg_p = pAgg.tile([128, ppe * NPER], f32, tag="agg")
            r2v = r2t[:].rearrange("p (a j i) -> p a j i", j=NPER, i=NPER)
            dg = r2t[:].rearrange("p (a q) -> p a q", q=GRID)[:, :, ::17]
            mm(agg_p[:], W["I15N"][:], dg, start=True, stop=False)
            for i in range(NPER):
                mm(agg_p[:], W["I15"][:], r2v[:, :, :, i],
                   start=False, stop=(i == NPER - 1))
            drain(AGGS[:, t * ppe * NPER:(t + 1) * ppe * NPER], agg_p[:])
            if step < NSTEPS - 1:
                nc.sync.dma_start(
                    scr_out[:, t * ppe * GRID:(t + 1) * ppe * GRID], r2t[:])

        # --- per-graph means
        with nc.allow_low_precision(reason="f32r view of f32 accumulate"):
            nc.vector.tensor_reduce(
                EMRS[:], AGGS[:].rearrange("p (g j) -> p g j", j=NPER),
                axis=mybir.AxisListType.X, op=AOT.add)

        # --- node loop
        nb1 = W[f"{m}_nb1_s23" if s23 else f"{m}_nb1_s1"]
        for nt in range(n_nt):
            ncol = npairs_nt * NPER
            nsl = slice(nt * ncol, (nt + 1) * ncol)
            nl = pSm.tile([128, 128], f32, tag="sm")
            mm(nl[:, :ncol], W[f"{m}_NX"][:], Xcur[:, nsl], start=True, stop=False)
            mm(nl[:, :ncol], W[f"{m}_NAGG"][:], AGGS[:, nsl], start=False,
               stop=False)
            mm(nl[:, :ncol], CnNS[:P, :],
               W["SELN"][:P, nt * ncol:(nt + 1) * ncol], start=False, stop=True)
            nr1 = r1p.tile([128, 128], DT, tag="nr1")
            drain(nr1[:, :ncol], nl[:, :ncol], bias=nb1, relu=True)
            nl2 = pSm.tile([128, 128], f32, tag="sm")
            mm(nl2[:, :ncol], W[f"{m}_NW2"][:], nr1[:, :ncol], start=True,
               stop=True)
            nr2 = r1p.tile([128, 128], DT, tag="nr2")
            drain(nr2[:, :ncol], nl2[:, :ncol], bias=W[f"{m}_nb2"], relu=True)
            nl3 = pSm.tile([128, 128], f32, tag="sm")
            mm(nl3[:, :ncol], W[f"{m}_NW3"][:], nr2[:, :ncol], start=True,
               stop=True)
            drain(Xnew[:, nsl], nl3[:, :ncol])

        with nc.allow_low_precision(reason="f32r view of f32 accumulate"):
            nc.vector.tensor_reduce(
                XMRS[:], Xnew[:].rearrange("p (g j) -> p g j", j=NPER),
                axis=mybir.AxisListType.X, op=AOT.add)

        # --- global MLP
        gl = pSm.tile([128, P], f32, tag="sm")
        mm(gl[:], W[f"{m}_GEM"][:], EMRS[:], start=True, stop=False)
        mm(gl[:], W[f"{m}_GXM"][:], XMRS[:], start=False, stop=False)
        for k, (ut, _, _, gu) in enumerate(Uins):
            mm(gl[:], W[gu][:], ut[:], start=False, stop=(k == len(Uins) - 1))
        gr1 = r1p.tile([128, P], DT, tag="gr1")
        drain(gr1[:], gl[:], bias=W[f"{m}_gb1"], relu=True)
        gl2 = pSm.tile([128, P], f32, tag="sm")
        mm(gl2[:], W[f"{m}_GW2"][:], gr1[:], start=True, stop=True)
        gr2 = r1p.tile([128, P], DT, tag="gr2")
        drain(gr2[:], gl2[:], bias=W[f"{m}_gb2"], relu=True)
        gl3 = pSm.tile([128, P], f32, tag="sm")
        mm(gl3[:], W[f"{m}_GW3"][:], gr2[:], start=True, stop=True)
        drain(Unew_t[:], gl3[:], bias=W[f"{m}_gb3"])

    # ----------------------------------------------------------- step loop
    for step in range(NSTEPS):
        cur, nxt = step % 2, (step + 1) % 2
        # meta-1 on query graph
        e_in = e1g[:].bitcast(DT) if step == 0 else scr[(1, "ab"[cur])][:]
        meta(step, 1, "m1", X[1][cur], X[1][nxt],
             [(U1[cur], "m1_CU", "m1_NU", "m1_GU")],
             U1new, e_in, scr[(1, "ab"[nxt])])
        # meta-2 on world graph, conditioned on U1new
        e_in2 = e2g[:].bitcast(DT) if step == 0 else scr[(2, "ab"[cur])][:]
        meta(step, 2, "m2", X[2][cur], X[2][nxt],
             [(U1new, "m2_CU1", "m2_NU1", "m2_GU1"),
              (U2[cur], "m2_CU2", "m2_NU2", "m2_GU2")],
             U2[nxt], e_in2, scr[(2, "ab"[nxt])])

        # u1 <- relu(u1new) @ resize_w + resize_b
        nc.vector.tensor_scalar(U1rel[:], U1new[:], 0.0, None, AOT.max)
        rs_p = pSm.tile([128, P], f32, tag="sm")
        mm(rs_p[:], W["RSZ"][:], U1rel[:], start=True, stop=True)
        drain(U1[nxt][:], rs_p[:], bias=W["rsb"])

        # readout from u2new
        ro1_p = pSm.tile([128, P], f32, tag="sm")
        mm(ro1_p[:], W["RO1"][:], U2[nxt][:], start=True, stop=True)
        ror1 = r1p.tile([128, P], DT, tag="ror1")
        drain(ror1[:], ro1_p[:], bias=W["rob1"], relu=True)
        ro2_p = pSm.tile([128, P], f32, tag="sm")
        mm(ro2_p[:], W["RO2"][:], ror1[:], start=True, stop=True)
        ror2 = r1p.tile([128, P], DT, tag="ror2")
        drain(ror2[:], ro2_p[:], bias=W["rob2"], relu=True)
        ro3_p = pSm.tile([2 * F_OUT, P], f32, tag="sm")
        mm(ro3_p[:], W["RO3"][:], ror2[:], start=True, stop=True)
        nc.vector.tensor_scalar(OUT[:, step * P:(step + 1) * P], ro3_p[:],
                                W["rob3"][:, 0:1], None, AOT.add)

    nc.sync.dma_start(out_d[:], OUT[:])
    return io


# ------------------------------------------------------------- entry points
_CACHE = {}


def get_program(P=P_FULL):
    if P in _CACHE:
        return _CACHE[P]
    nc = bacc.Bacc("TRN2", target_bir_lowering=False, debug=False,
                   num_devices=NCORES)
    with tile.TileContext(nc) as tc:
        with ExitStack() as ctx:
            build_kernel(nc, tc, ctx, P)
    nc.compile()
    _CACHE[P] = nc
    return nc


def host_pack(inputs, n_cores=NCORES):
    C = make_consts(inputs["edge_q"], inputs["node_q"], inputs["global_q"],
                    inputs["edge_w"], inputs["node_w"], inputs["global_w"],
                    inputs["resize_w"], inputs["resize_b"], inputs["readout"])
    e1 = pack_edges(np.asarray(inputs["e1"], np.float32), n_cores)
    e2 = pack_edges(np.asarray(inputs["e2"], np.float32), n_cores)
    x1 = pack_nodes(np.asarray(inputs["x1"], np.float32), n_cores)
    x2 = pack_nodes(np.asarray(inputs["x2"], np.float32), n_cores)
    u1 = pack_globals(np.asarray(inputs["u1"], np.float32), n_cores)
    u2 = pack_globals(np.asarray(inputs["u2"], np.float32), n_cores)
    maps = []
    for c in range(n_cores):
        m = {"e1g": e1[c], "e2g": e2[c], "x1g": x1[c], "x2g": x2[c],
             "u1g": u1[c], "u2g": u2[c]}
        for k, v in C.items():
            m[k] = v
        maps.append(m)
    return maps


def host_unpack(outs, n_cores=NCORES):
    """per-core [4, 3P] -> [NSTEPS, nb, F_OUT]."""
    P = outs[0].shape[1] // NSTEPS
    gpc = 2 * P
    y = np.zeros((NSTEPS, gpc * n_cores, F_OUT), np.float32)
    for c, o in enumerate(outs):
        o = o.reshape(2, F_OUT, NSTEPS, P)       # [par, f, s, p]
        for par in range(2):
            y[:, c * gpc + 2 * np.arange(P) + par, :] = \
                o[par].transpose(1, 2, 0)        # [s, p, f]
    return y


def kernel(x1, e1, u1, x2, e2, u2, edge_index1, batch1, edge_index2, batch2,
           edge_q, node_q, global_q, edge_w, node_w, global_w,
           resize_w, resize_b, readout):
    inputs = dict(x1=x1, e1=e1, u1=u1, x2=x2, e2=e2, u2=u2,
                  edge_q=edge_q, node_q=node_q, global_q=global_q,
                  edge_w=edge_w, node_w=node_w, global_w=global_w,
                  resize_w=resize_w, resize_b=resize_b, readout=readout)
    nc = get_program()
    in_maps = host_pack(inputs)
    res = run_bass_kernel_spmd(nc, in_maps, core_ids=list(range(NCORES)))
    outs = [res.results[c]["out"] for c in range(NCORES)]
    return host_unpack(outs)


# revision 14
# speedup vs baseline: 1.2663x; 1.2663x over previous
"""Trainium2 Bass kernel for a 3-step conditioned GraphNets MetaLayer pair.

Problem structure (hardcoded, matches the generator):
  B=2048 graphs, 16 nodes/graph (block-contiguous), complete digraph per
  graph (240 edges, src-major lexicographic order), H=HU=64, 3 steps.

Strategy: data-parallel over graphs, 256 graphs/core on 8 cores.
On-device layout is "graph-paired transposed": 128 SBUF partitions hold
features of two consecutive graphs (rows 0:64 = even graph, 64:128 = odd
graph); columns enumerate edges/nodes/graphs.  Edges use a per-graph
256-slot grid (slot = 16*dst_local + src_local, diagonal unused) so that
gathers/scatters become structured access patterns:
  - x[dst]-x[src] terms become a matmul against a constant +/-1 selector
  - per-node incoming-edge means become 16 strided accumulating matmuls
  - per-graph means become pool_avg over contiguous windows
The inter-step edge state stored to HBM is R2 = relu(hidden2) with the
last MLP layer (W3, b3) folded into every consumer's weights/biases, which
removes a full matmul pass and a full PSUM-drain pass per meta-step.
All matmuls run as fp32r (TF32-like) by default.
"""

import sys
import os
from contextlib import ExitStack

sys.path.insert(0, "/opt/trn_rl_repo")

import numpy as np

import concourse.bass as bass
import concourse.bacc as bacc
import concourse.tile as tile
from concourse import mybir
from concourse.bass_utils import run_bass_kernel_spmd

# ----------------------------------------------------------------- constants
B = 2048
NPER = 16
H = 64
EPG = NPER * (NPER - 1)      # 240
GRID = NPER * NPER           # 256
NSTEPS = 3
F_OUT = 2
NCORES = 8
GPC = B // NCORES            # graphs per core = 256
P_FULL = GPC // 2            # pairs per core = 128

f32 = mybir.dt.float32
f32r = mybir.dt.float32r
bf16 = mybir.dt.bfloat16

AOT = mybir.AluOpType
AFT = mybir.ActivationFunctionType

# compute/storage dtype for activations & weights on device
import ml_dtypes
MODE = os.environ.get("KMODE", "bf16")
if MODE == "bf16":
    DT = bf16
    DT_NP = ml_dtypes.bfloat16
    PSUM_DT = f32            # bass requires fp32 matmul outputs
    CHUNK = 512
else:
    DT = f32r
    DT_NP = np.float32
    PSUM_DT = f32
    CHUNK = 512
PPE = 16                     # pairs per edge tile (AGG group)


# ------------------------------------------------------- host index helpers
def _grid_slots():
    """packed edge order -> grid slot (16*dst + src)."""
    slots = []
    for i in range(NPER):
        for j in range(NPER):
            if i != j:
                slots.append(16 * j + i)
    return np.asarray(slots, np.int64)


_SLOTS = _grid_slots()


def pack_edges(e, n_cores=NCORES):
    """[nb*240, H] -> per-core [128, (nb/n_cores/2)*256], paired grid."""
    nb = e.shape[0] // EPG
    gpc = nb // n_cores
    eg = np.zeros((nb, GRID, H), np.float32)
    eg[:, _SLOTS, :] = e.reshape(nb, EPG, H)
    out = []
    for c in range(n_cores):
        g = eg[c * gpc:(c + 1) * gpc]                       # [gpc, 256, H]
        g = g.reshape(gpc // 2, 2, GRID, H).transpose(1, 3, 0, 2)
        out.append(np.ascontiguousarray(g.reshape(2 * H, (gpc // 2) * GRID).astype(DT_NP)))
    return out


def pack_nodes(x, n_cores=NCORES):
    """[nb*16, H] -> per-core [128, (nb/n_cores/2)*16]."""
    nb = x.shape[0] // NPER
    gpc = nb // n_cores
    out = []
    for c in range(n_cores):
        g = x.reshape(nb, NPER, H)[c * gpc:(c + 1) * gpc]
        g = g.reshape(gpc // 2, 2, NPER, H).transpose(1, 3, 0, 2)
        out.append(np.ascontiguousarray(g.reshape(2 * H, (gpc // 2) * NPER).astype(DT_NP)))
    return out


def pack_globals(u, n_cores=NCORES):
    """[nb, H] -> per-core [128, nb/n_cores/2]."""
    gpc = u.shape[0] // n_cores
    out = []
    for c in range(n_cores):
        g = u[c * gpc:(c + 1) * gpc]
        g = g.reshape(gpc // 2, 2, H).transpose(1, 2, 0)
        out.append(np.ascontiguousarray(g.reshape(2 * H, gpc // 2).astype(DT_NP)))
    return out


def bd2(w):
    """[64, m] -> [128, 2m] block-diagonal (paired lhsT)."""
    k, m = w.shape
    o = np.zeros((2 * k, 2 * m), np.float32)
    o[:k, :m] = w
    o[k:, m:] = w
    return o


def pvec(b):
    """[64] -> [128, 1] paired bias column."""
    return np.concatenate([b, b]).reshape(2 * H, 1).astype(np.float32)


def sel_y():
    """[128, 2048] 8-pair block-diag +/-1 Y selector (rows = 8x16 nodes)."""
    s = np.zeros((NPER, GRID), np.float32)
    for q in range(GRID):
        j, i = q // 16, q % 16
        s[j, q] += 1.0
        s[i, q] -= 1.0
    out = np.zeros((128, 8 * GRID), np.float32)
    for m in range(8):
        out[16 * m:16 * m + 16, GRID * m:GRID * m + GRID] = s
    return out


def sel_n():
    """[128, 2048] pair -> its 16 node-columns broadcast selector."""
    s = np.zeros((128, 128 * NPER), np.float32)
    for r in range(128):
        s[r, 16 * r:16 * r + 16] = 1.0
    return s


def make_consts(edge_q, node_q, global_q, edge_w, node_w, global_w,
                resize_w, resize_b, readout):
    """Host-precomputed constant tensors (shared by all cores)."""
    def unpack(mlp):
        (w1, b1), (w2, b2), (w3, b3) = mlp
        return (np.asarray(w1, np.float32), np.asarray(b1, np.float32),
                np.asarray(w2, np.float32), np.asarray(b2, np.float32),
                np.asarray(w3, np.float32), np.asarray(b3, np.float32))

    eW1, eb1, eW2, eb2, eW3, eb3 = unpack(edge_q)
    nW1, nb1, nW2, nb2, nW3, nb3 = unpack(node_q)
    gW1, gb1, gW2, gb2, gW3, gb3 = unpack(global_q)
    e2W1, e2b1, e2W2, e2b2, e2W3, e2b3 = unpack(edge_w)
    n2W1, n2b1, n2W2, n2b2, n2W3, n2b3 = unpack(node_w)
    g2W1, g2b1, g2W2, g2b2, g2W3, g2b3 = unpack(global_w)
    roW1, rob1, roW2, rob2, roW3, rob3 = unpack(readout)
    rsW = np.asarray(resize_w, np.float32)
    rsb = np.asarray(resize_b, np.float32)

    C = {}
    # ---- meta-1 (query graph), edge_q input = [e, dx, u]
    W1e, W1dx, W1u = eW1[0:64], eW1[64:128], eW1[128:192]
    C["m1_A_s1"] = bd2(W1e)
    C["m1_A_s23"] = bd2(eW3 @ W1e)
    C["m1_DX"] = bd2(W1dx)
    C["m1_CU"] = bd2(W1u)
    C["m1_W2"] = bd2(eW2)
    C["m1_cb_s1"] = pvec(eb1)
    C["m1_cb_s23"] = pvec(eb1 + W1e.T @ eb3)
    C["m1_b2"] = pvec(eb2)
    # node_q input = [x, agg, u]
    Wnx, Wnagg, Wnu = nW1[0:64], nW1[64:128], nW1[128:192]
    C["m1_NX"] = bd2(Wnx)
    C["m1_NAGG"] = bd2(eW3 @ Wnagg / 15.0)
    C["m1_NU"] = bd2(Wnu)
    C["m1_NW2"] = bd2(nW2)
    C["m1_NW3"] = bd2(nW3)
    C["m1_nb1_s1"] = pvec(nb1 + Wnagg.T @ eb3)
    C["m1_nb1_s23"] = pvec(nb1 + Wnagg.T @ eb3 + Wnx.T @ nb3)
    C["m1_nb2"] = pvec(nb2)
    # global_q input = [em, xm, u]
    Wgem, Wgxm, Wgu = gW1[0:64], gW1[64:128], gW1[128:192]
    C["m1_GEM"] = bd2(eW3 @ Wgem / (16.0 * 15.0))
    C["m1_GXM"] = bd2(Wgxm / 16.0)
    C["m1_GU"] = bd2(Wgu)
    C["m1_GW2"] = bd2(gW2)
    C["m1_GW3"] = bd2(gW3)
    C["m1_gb1"] = pvec(gb1 + Wgem.T @ eb3 + Wgxm.T @ nb3)
    C["m1_gb2"] = pvec(gb2)
    C["m1_gb3"] = pvec(gb3)

    # ---- meta-2 (world graph), edge_w input = [e2c, dx2c, u2c]
    #   e2c=[e2; u1], dx2c=[dx2; 0], u2c=[u2; u1]
    W1e2 = e2W1[0:64]
    Wc_u1 = e2W1[64:128] + e2W1[320:384]
    W1dx2 = e2W1[128:192]
    Wc_u2 = e2W1[256:320]
    C["m2_A_s1"] = bd2(W1e2)
    C["m2_A_s23"] = bd2(e2W3 @ W1e2)
    C["m2_DX"] = bd2(W1dx2)
    C["m2_CU1"] = bd2(Wc_u1)
    C["m2_CU2"] = bd2(Wc_u2)
    C["m2_W2"] = bd2(e2W2)
    C["m2_cb_s1"] = pvec(e2b1)
    C["m2_cb_s23"] = pvec(e2b1 + W1e2.T @ e2b3)
    C["m2_b2"] = pvec(e2b2)
    # node_w input = [x2c, agg2, u2c] = [[x2; u1], agg2, [u2; u1]]
    Wnx2 = n2W1[0:64]
    Wn2_u1 = n2W1[64:128] + n2W1[256:320]
    Wnagg2 = n2W1[128:192]
    Wn2_u2 = n2W1[192:256]
    C["m2_NX"] = bd2(Wnx2)
    C["m2_NAGG"] = bd2(e2W3 @ Wnagg2 / 15.0)
    C["m2_NU1"] = bd2(Wn2_u1)
    C["m2_NU2"] = bd2(Wn2_u2)
    C["m2_NW2"] = bd2(n2W2)
    C["m2_NW3"] = bd2(n2W3)
    C["m2_nb1_s1"] = pvec(n2b1 + Wnagg2.T @ e2b3)
    C["m2_nb1_s23"] = pvec(n2b1 + Wnagg2.T @ e2b3 + Wnx2.T @ n2b3)
    C["m2_nb2"] = pvec(n2b2)
    # global_w input = [em2, xm2, u2c]
    Wgem2, Wgxm2 = g2W1[0:64], g2W1[64:128]
    Wg2_u2, Wg2_u1 = g2W1[128:192], g2W1[192:256]
    C["m2_GEM"] = bd2(e2W3 @ Wgem2 / (16.0 * 15.0))
    C["m2_GXM"] = bd2(Wgxm2 / 16.0)
    C["m2_GU1"] = bd2(Wg2_u1)
    C["m2_GU2"] = bd2(Wg2_u2)
    C["m2_GW2"] = bd2(g2W2)
    C["m2_GW3"] = bd2(g2W3)
    C["m2_gb1"] = pvec(g2b1 + Wgem2.T @ e2b3 + Wgxm2.T @ n2b3)
    C["m2_gb2"] = pvec(g2b2)
    C["m2_gb3"] = pvec(g2b3)

    # ---- tail
    C["RSZ"] = bd2(rsW)
    C["rsb"] = pvec(rsb)
    C["RO1"] = bd2(roW1)
    C["RO2"] = bd2(roW2)
    C["RO3"] = bd2(roW3)                       # [128, 4]
    C["rob1"] = pvec(rob1)
    C["rob2"] = pvec(rob2)
    C["rob3"] = np.concatenate([rob3, rob3]).reshape(2 * F_OUT, 1).astype(np.float32)
    C["SELY"] = sel_y()
    C["SELN"] = sel_n()
    C["IDT"] = np.eye(128).astype(np.float32)
    C["IDTN"] = (-np.eye(128)).astype(np.float32)
    return C


# --------------------------------------------------------------- the kernel
def build_kernel(nc, tc, ctx, P):
    """Emit the full 3-step program for one core handling 2P graphs."""
    npairs_nt = min(8, P)            # pairs per node tile
    ppe = min(PPE, P)                # pairs per edge tile
    n_et = P // ppe
    n_nt = P // npairs_nt
    chunks_per_et = (ppe * GRID) // CHUNK
    pairs_per_chunk = CHUNK // GRID  # 2

    io = {}

    def ext_in(name, shape, dtype=f32):
        io[name] = nc.dram_tensor(name, list(shape), dtype, kind="ExternalInput").ap()
        return io[name]

    data_dt = bf16 if MODE == "bf16" else f32

    def din(ap):
        # view a data input as the matmul dtype
        return ap[:] if MODE == "bf16" else ap[:].bitcast(f32r)

    # inputs
    e1g = ext_in("e1g", (128, P * GRID), data_dt)
    e2g = ext_in("e2g", (128, P * GRID), data_dt)
    x1g = ext_in("x1g", (128, P * NPER), data_dt)
    x2g = ext_in("x2g", (128, P * NPER), data_dt)
    u1g = ext_in("u1g", (128, P), data_dt)
    u2g = ext_in("u2g", (128, P), data_dt)
    # const inputs (names must match make_consts keys)
    const_shapes = {
        "m1_A_s1": (128, 128), "m1_A_s23": (128, 128), "m1_DX": (128, 128),
        "m1_CU": (128, 128), "m1_W2": (128, 128),
        "m1_cb_s1": (128, 1), "m1_cb_s23": (128, 1), "m1_b2": (128, 1),
        "m1_NX": (128, 128), "m1_NAGG": (128, 128), "m1_NU": (128, 128),
        "m1_NW2": (128, 128), "m1_NW3": (128, 128),
        "m1_nb1_s1": (128, 1), "m1_nb1_s23": (128, 1), "m1_nb2": (128, 1),
        "m1_GEM": (128, 128), "m1_GXM": (128, 128), "m1_GU": (128, 128),
        "m1_GW2": (128, 128), "m1_GW3": (128, 128),
        "m1_gb1": (128, 1), "m1_gb2": (128, 1), "m1_gb3": (128, 1),
        "m2_A_s1": (128, 128), "m2_A_s23": (128, 128), "m2_DX": (128, 128),
        "m2_CU1": (128, 128), "m2_CU2": (128, 128), "m2_W2": (128, 128),
        "m2_cb_s1": (128, 1), "m2_cb_s23": (128, 1), "m2_b2": (128, 1),
        "m2_NX": (128, 128), "m2_NAGG": (128, 128), "m2_NU1": (128, 128),
        "m2_NU2": (128, 128), "m2_NW2": (128, 128), "m2_NW3": (128, 128),
        "m2_nb1_s1": (128, 1), "m2_nb1_s23": (128, 1), "m2_nb2": (128, 1),
        "m2_GEM": (128, 128), "m2_GXM": (128, 128), "m2_GU1": (128, 128),
        "m2_GU2": (128, 128), "m2_GW2": (128, 128), "m2_GW3": (128, 128),
        "m2_gb1": (128, 1), "m2_gb2": (128, 1), "m2_gb3": (128, 1),
        "RSZ": (128, 128), "rsb": (128, 1),
        "RO1": (128, 128), "RO2": (128, 128), "RO3": (128, 4),
        "rob1": (128, 1), "rob2": (128, 1), "rob3": (2 * F_OUT, 1),
        "SELY": (128, 8 * GRID), "SELN": (128, 128 * NPER),
        "IDT": (128, 128), "IDTN": (128, 128),
    }
    for nm, shp in const_shapes.items():
        ext_in(nm, shp, f32 if (shp[1] == 1 or MODE != "bf16") else bf16)

    out_d = nc.dram_tensor("out", [2 * F_OUT, NSTEPS * P], f32,
                           kind="ExternalOutput").ap()
    io["out"] = out_d

    # scratch (internal DRAM), ping-pong per edge family
    scr = {(fam, pp): nc.dram_tensor(f"scr{fam}{pp}", [128, P * GRID], DT).ap()
           for fam in (1, 2) for pp in ("a", "b")}

    # ------------------------------------------------------------ SBUF pools
    res = ctx.enter_context(tc.tile_pool(name="res", bufs=1))
    wpool = ctx.enter_context(tc.tile_pool(name="w", bufs=1))
    etp = ctx.enter_context(tc.tile_pool(name="et", bufs=3))
    r1p = ctx.enter_context(tc.tile_pool(name="r1", bufs=3))
    nb_l2 = 2 if CHUNK == 512 else 1
    pL1 = ctx.enter_context(tc.tile_pool(name="pL1", bufs=2, space="PSUM"))
    pL2 = ctx.enter_context(tc.tile_pool(name="pL2", bufs=nb_l2, space="PSUM"))
    pAgg = ctx.enter_context(tc.tile_pool(name="pAgg", bufs=nb_l2, space="PSUM"))
    pSm = ctx.enter_context(tc.tile_pool(name="pSm", bufs=nb_l2, space="PSUM"))

    # load constants to SBUF
    W = {}
    for nm, shp in const_shapes.items():
        dt_ = f32 if shp[1] == 1 else DT
        t = wpool.tile(list(shp), dt_, tag=nm, name="w_" + nm)
        src = io[nm]
        nc.sync.dma_start(t[:], src[:].bitcast(dt_) if dt_ != f32 else src[:])
        W[nm] = t

    # resident tiles
    def rtile(name, cols, dt_=DT):
        return res.tile([128, cols], dt_, tag=name, name=name)

    X = {1: [rtile("x1a", P * NPER), rtile("x1b", P * NPER)],
         2: [rtile("x2a", P * NPER), rtile("x2b", P * NPER)]}
    U1 = [rtile("u1a", P), rtile("u1b", P)]
    U2 = [rtile("u2a", P), rtile("u2b", P)]
    U1new = rtile("u1new", P)
    U1rel = rtile("u1rel", P)
    YS = rtile("ys", n_nt * 128)
    CTS = rtile("cts", P, f32)
    CnNS = rtile("cnns", 128)
    AGGS = rtile("aggs", P * NPER)
    EMRS = rtile("emrs", P)
    XMRS = rtile("xmrs", P)
    R2 = [rtile("r2a", ppe * GRID), rtile("r2b", ppe * GRID)]
    OUT = res.tile([2 * F_OUT, NSTEPS * P], f32, tag="outt", name="outt")

    # initial loads
    nc.sync.dma_start(X[1][0][:], din(x1g))
    nc.sync.dma_start(X[2][0][:], din(x2g))
    nc.sync.dma_start(U1[0][:], din(u1g))
    nc.sync.dma_start(U2[0][:], din(u2g))

    mm = nc.tensor.matmul
    drain_ct = [0]

    def drain(out_ap, in_ap, bias=None, relu=False):
        """PSUM -> SBUF with optional per-partition bias and relu.
        Alternates DVE / ACT for throughput."""
        use_act = (drain_ct[0] % 2 == 1) and relu
        drain_ct[0] += 1
        if use_act:
            nc.scalar.activation(out_ap, in_ap, AFT.Relu,
                                 bias=(bias[:, 0:1] if bias is not None else 0.0),
                                 scale=1.0)
        else:
            s1 = bias[:, 0:1] if bias is not None else 0.0
            if relu:
                nc.vector.tensor_scalar(out_ap, in_ap, s1, 0.0, AOT.add, AOT.max)
            else:
                nc.vector.tensor_scalar(out_ap, in_ap, s1, None, AOT.add)

    # --------------------------------------------------------------- 1 meta
    def meta(step, fam, m, Xcur, Xnew, Uins, Unew_t, egrid_in, scr_out):
        """One MetaLayer. m = 'm1'/'m2'; Uins = list of (Utile, CUw, NUw, GUw)."""
        s23 = step > 0
        # --- CTS: per-pair edge constant term (transposed) + bias
        cts_p = pSm.tile([128, P], f32, tag="sm")
        for k, (ut, cu, _, _) in enumerate(Uins):
            mm(cts_p[:], W[cu][:], ut[:], start=(k == 0),
               stop=(k == len(Uins) - 1))
        drain(CTS[:], cts_p[:], bias=W[f"{m}_cb_s23" if s23 else f"{m}_cb_s1"])

        # --- Cn (natural, per-pair node constant term)
        cn_p = pSm.tile([128, 128], f32, tag="sm")
        for k, (ut, _, nu, _) in enumerate(Uins):
            mm(cn_p[:P, :], ut[:], W[nu][:], start=(k == 0),
               stop=(k == len(Uins) - 1))
        drain(CnNS[:P, :], cn_p[:P, :])

        # --- Y tiles (natural paired: rows = nodes of 8 pairs, cols = feats)
        for nt in range(n_nt):
            ncol = npairs_nt * NPER
            y_p = pSm.tile([128, 128], f32, tag="sm")
            mm(y_p[:ncol, :], Xcur[:, nt * ncol:(nt + 1) * ncol],
               W[f"{m}_DX"][:], start=True, stop=True)
            drain(YS[:ncol, nt * 128:nt * 128 + 128], y_p[:ncol, :])

        # --- edge loop
        A = W[f"{m}_A_s23" if s23 else f"{m}_A_s1"]
        for t in range(n_et):
            et = etp.tile([128, ppe * GRID], DT, tag="et")
            nc.sync.dma_start(et[:], egrid_in[:, t * ppe * GRID:(t + 1) * ppe * GRID])
            r2t = R2[t % 2]
            for g in range(chunks_per_et // 2):
                cps = [2 * g, 2 * g + 1]
                sls = [slice(c * CHUNK, (c + 1) * CHUNK) for c in cps]
                p0s = [t * ppe + c * pairs_per_chunk for c in cps]
                l1s = [pL1.tile([128, CHUNK], f32, tag="l1", name="l1")
                       for _ in cps]
                # same-lhsT matmuls back to back: one weight load each
                for l1, sl in zip(l1s, sls):
                    mm(l1[:], A[:], et[:, sl], start=True, stop=False)
                krows = 16 * npairs_nt
                for l1, pair0 in zip(l1s, p0s):
                    nt = pair0 // npairs_nt
                    ysl = YS[:krows, nt * 128:nt * 128 + 128]
                    selc = (pair0 % npairs_nt) * GRID
                    mm(l1[:], ysl, W["SELY"][:krows, selc:selc + CHUNK],
                       start=False, stop=True)
                r1s = [r1p.tile([128, CHUNK], DT, tag="r1", name="r1")
                       for _ in cps]
                for l1, r1, pair0 in zip(l1s, r1s, p0s):
                    for pp in range(pairs_per_chunk):
                        psl = slice(pp * GRID, (pp + 1) * GRID)
                        drain(r1[:, psl], l1[:, psl],
                              bias=CTS[:, pair0 + pp:pair0 + pp + 1], relu=True)
                l2s = [pL2.tile([128, CHUNK], f32, tag="l2", name="l2")
                       for _ in cps]
                for l2, r1 in zip(l2s, r1s):
                    mm(l2[:], W[f"{m}_W2"][:], r1[:], start=True, stop=True)
                for l2, sl in zip(l2s, sls):
                    drain(r2t[:, sl], l2[:], bias=W[f"{m}_b2"], relu=True)
            # aggregation over this tile's ppe pairs (means w/ diag excluded)
            agg_p = pAgg.tile([128, ppe * NPER], f32, tag="agg")
            r2v = r2t[:].rearrange("p (a j i) -> p a j i", j=NPER, i=NPER)
            dg = r2t[:].rearrange("p (a q) -> p a q", q=GRID)[:, :, ::17]
            for i in range(NPER):
                mm(agg_p[:], W["IDT"][:], r2v[:, :, :, i],
                   start=(i == 0), stop=False)
            mm(agg_p[:], W["IDTN"][:], dg, start=False, stop=True)
            drain(AGGS[:, t * ppe * NPER:(t + 1) * ppe * NPER], agg_p[:])
            if step < NSTEPS - 1:
                nc.sync.dma_start(
                    scr_out[:, t * ppe * GRID:(t + 1) * ppe * GRID], r2t[:])

        # --- per-graph means
        with nc.allow_low_precision(reason="f32r view of f32 accumulate"):
            nc.vector.tensor_reduce(
                EMRS[:], AGGS[:].rearrange("p (g j) -> p g j", j=NPER),
                axis=mybir.AxisListType.X, op=AOT.add)

        # --- node loop
        nb1 = W[f"{m}_nb1_s23" if s23 else f"{m}_nb1_s1"]
        for nt in range(n_nt):
            ncol = npairs_nt * NPER
            nsl = slice(nt * ncol, (nt + 1) * ncol)
            nl = pSm.tile([128, 128], f32, tag="sm")
            mm(nl[:, :ncol], W[f"{m}_NX"][:], Xcur[:, nsl], start=True, stop=False)
            mm(nl[:, :ncol], W[f"{m}_NAGG"][:], AGGS[:, nsl], start=False,
               stop=False)
            mm(nl[:, :ncol], CnNS[:P, :],
               W["SELN"][:P, nt * ncol:(nt + 1) * ncol], start=False, stop=True)
            nr1 = r1p.tile([128, 128], DT, tag="nr1")
            drain(nr1[:, :ncol], nl[:, :ncol], bias=nb1, relu=True)
            nl2 = pSm.tile([128, 128], f32, tag="sm")
            mm(nl2[:, :ncol], W[f"{m}_NW2"][:], nr1[:, :ncol], start=True,
               stop=True)
            nr2 = r1p.tile([128, 128], DT, tag="nr2")
            drain(nr2[:, :ncol], nl2[:, :ncol], bias=W[f"{m}_nb2"], relu=True)
            nl3 = pSm.tile([128, 128], f32, tag="sm")
            mm(nl3[:, :ncol], W[f"{m}_NW3"][:], nr2[:, :ncol], start=True,
               stop=True)
            drain(Xnew[:, nsl], nl3[:, :ncol])

        with nc.allow_low_precision(reason="f32r view of f32 accumulate"):
            nc.vector.tensor_reduce(
                XMRS[:], Xnew[:].rearrange("p (g j) -> p g j", j=NPER),
                axis=mybir.AxisListType.X, op=AOT.add)

        # --- global MLP
        gl = pSm.tile([128, P], f32, tag="sm")
        mm(gl[:], W[f"{m}_GEM"][:], EMRS[:], start=True, stop=False)
        mm(gl[:], W[f"{m}_GXM"][:], XMRS[:], start=False, stop=False)
        for k, (ut, _, _, gu) in enumerate(Uins):
            mm(gl[:], W[gu][:], ut[:], start=False, stop=(k == len(Uins) - 1))
        gr1 = r1p.tile([128, P], DT, tag="gr1")
        drain(gr1[:], gl[:], bias=W[f"{m}_gb1"], relu=True)
        gl2 = pSm.tile([128, P], f32, tag="sm")
        mm(gl2[:], W[f"{m}_GW2"][:], gr1[:], start=True, stop=True)
        gr2 = r1p.tile([128, P], DT, tag="gr2")
        drain(gr2[:], gl2[:], bias=W[f"{m}_gb2"], relu=True)
        gl3 = pSm.tile([128, P], f32, tag="sm")
        mm(gl3[:], W[f"{m}_GW3"][:], gr2[:], start=True, stop=True)
        drain(Unew_t[:], gl3[:], bias=W[f"{m}_gb3"])

    # ----------------------------------------------------------- step loop
    for step in range(NSTEPS):
        cur, nxt = step % 2, (step + 1) % 2
        # meta-1 on query graph
        e_in = din(e1g) if step == 0 else scr[(1, "ab"[cur])][:]
        meta(step, 1, "m1", X[1][cur], X[1][nxt],
             [(U1[cur], "m1_CU", "m1_NU", "m1_GU")],
             U1new, e_in, scr[(1, "ab"[nxt])])
        # meta-2 on world graph, conditioned on U1new
        e_in2 = din(e2g) if step == 0 else scr[(2, "ab"[cur])][:]
        meta(step, 2, "m2", X[2][cur], X[2][nxt],
             [(U1new, "m2_CU1", "m2_NU1", "m2_GU1"),
              (U2[cur], "m2_CU2", "m2_NU2", "m2_GU2")],
             U2[nxt], e_in2, scr[(2, "ab"[nxt])])

        # u1 <- relu(u1new) @ resize_w + resize_b
        nc.vector.tensor_scalar(U1rel[:], U1new[:], 0.0, None, AOT.max)
        rs_p = pSm.tile([128, P], f32, tag="sm")
        mm(rs_p[:], W["RSZ"][:], U1rel[:], start=True, stop=True)
        drain(U1[nxt][:], rs_p[:], bias=W["rsb"])

        # readout from u2new
        ro1_p = pSm.tile([128, P], f32, tag="sm")
        mm(ro1_p[:], W["RO1"][:], U2[nxt][:], start=True, stop=True)
        ror1 = r1p.tile([128, P], DT, tag="ror1")
        drain(ror1[:], ro1_p[:], bias=W["rob1"], relu=True)
        ro2_p = pSm.tile([128, P], f32, tag="sm")
        mm(ro2_p[:], W["RO2"][:], ror1[:], start=True, stop=True)
        ror2 = r1p.tile([128, P], DT, tag="ror2")
        drain(ror2[:], ro2_p[:], bias=W["rob2"], relu=True)
        ro3_p = pSm.tile([2 * F_OUT, P], f32, tag="sm")
        mm(ro3_p[:], W["RO3"][:], ror2[:], start=True, stop=True)
        nc.vector.tensor_scalar(OUT[:, step * P:(step + 1) * P], ro3_p[:],
                                W["rob3"][:, 0:1], None, AOT.add)

    nc.sync.dma_start(out_d[:], OUT[:])
    return io


# ------------------------------------------------------------- entry points
_CACHE = {}


def get_program(P=P_FULL):
    if P in _CACHE:
        return _CACHE[P]
    nc = bacc.Bacc("TRN2", target_bir_lowering=False, debug=False,
                   num_devices=NCORES)
    with tile.TileContext(nc) as tc:
        with ExitStack() as ctx:
            build_kernel(nc, tc, ctx, P)
    nc.compile()
    _CACHE[P] = nc
    return nc


def host_pack(inputs, n_cores=NCORES):
    C = make_consts(inputs["edge_q"], inputs["node_q"], inputs["global_q"],
                    inputs["edge_w"], inputs["node_w"], inputs["global_w"],
                    inputs["resize_w"], inputs["resize_b"], inputs["readout"])
    e1 = pack_edges(np.asarray(inputs["e1"], np.float32), n_cores)
    e2 = pack_edges(np.asarray(inputs["e2"], np.float32), n_cores)
    x1 = pack_nodes(np.asarray(inputs["x1"], np.float32), n_cores)
    x2 = pack_nodes(np.asarray(inputs["x2"], np.float32), n_cores)
    u1 = pack_globals(np.asarray(inputs["u1"], np.float32), n_cores)
    u2 = pack_globals(np.asarray(inputs["u2"], np.float32), n_cores)
    maps = []
    for c in range(n_cores):
        m = {"e1g": e1[c], "e2g": e2[c], "x1g": x1[c], "x2g": x2[c],
             "u1g": u1[c], "u2g": u2[c]}
        for k, v in C.items():
            m[k] = v if (v.shape[1] == 1 or MODE != "bf16") else v.astype(DT_NP)
        maps.append(m)
    return maps


def host_unpack(outs, n_cores=NCORES):
    """per-core [4, 3P] -> [NSTEPS, nb, F_OUT]."""
    P = outs[0].shape[1] // NSTEPS
    gpc = 2 * P
    y = np.zeros((NSTEPS, gpc * n_cores, F_OUT), np.float32)
    for c, o in enumerate(outs):
        o = o.reshape(2, F_OUT, NSTEPS, P)       # [par, f, s, p]
        for par in range(2):
            y[:, c * gpc + 2 * np.arange(P) + par, :] = \
                o[par].transpose(1, 2, 0)        # [s, p, f]
    return y


def kernel(x1, e1, u1, x2, e2, u2, edge_index1, batch1, edge_index2, batch2,
           edge_q, node_q, global_q, edge_w, node_w, global_w,
           resize_w, resize_b, readout):
    inputs = dict(x1=x1, e1=e1, u1=u1, x2=x2, e2=e2, u2=u2,
                  edge_q=edge_q, node_q=node_q, global_q=global_q,
                  edge_w=edge_w, node_w=node_w, global_w=global_w,
                  resize_w=resize_w, resize_b=resize_b, readout=readout)
    nc = get_program()
    in_maps = host_pack(inputs)
    res = run_bass_kernel_spmd(nc, in_maps, core_ids=list(range(NCORES)))
    outs = [res.results[c]["out"] for c in range(NCORES)]
    return host_unpack(outs)
